# revision 3
# baseline (speedup 1.0000x reference)
"""BitLinear TRN2 kernel: out = (x @ ternary(W).T) * scale(W).

Reference semantics (fp32):
    absmean = mean(|W|, axis=1)                    # [O]
    ternary = sign(W) * (|W| > 0.7 * absmean)      # [O, I] in {-1, 0, +1}
    out     = (x @ ternary.T) * absmean            # [B, S, O]

Distribution: column-parallel (shard W rows = out-features) across 8 cores,
x replicated.  Each core computes outT_c = (ternary_c @ x.T) * scale_c, i.e.
the transposed output slice [O_c, M].  This keeps both the ternarization
threshold and the final scale as natural per-partition [P, 1] broadcasts.

On-chip per core:
  phase A: DMA W slice -> absmean (DVE reduce with |.|) -> thr -> ternary in
           bf16 via (w > thr) - (w < -thr) -> PE-transpose 128x128 tiles ->
           resident lhsT tiles (dtype per strategy).
  phase B: stream xT strips [128, MT] fp32, round to matmul dtype on DVE,
           accumulate psum[o=128, m=MT] over 32 k-tiles on the PE, copy back
           with per-partition scale on ACT, DMA out.

Matmul dtype strategy (STRATEGY):
  "f32r"  : single GEMM, fp32-reduced PE mode (1 cyc/row, ~1.6e-4 matmul err)
  "bf16x2": x split into bf16 hi+lo, two accumulated GEMMs (~1e-5 err, 2x PE)
  "bf16"  : single bf16 GEMM (fastest to same speed as f32r, ~2e-3 err)

Host side only reshapes/transposes (layout), all arithmetic is on-device.
"""

import os

import numpy as np

import concourse.bass as bass
import concourse.mybir as mybir
import concourse.tile as tile
from concourse import bacc
from concourse.bass_utils import run_bass_kernel_spmd
from concourse.masks import make_identity

ALPHA = 0.7
N_CORES = 8

# Full problem shapes (hardcoded per contract).
B, S, I, O = 8, 2048, 4096, 4096
M = B * S  # 16384 tokens

# Sharding grid: NO (out-feature shards) x NM (token shards), NO * NM = 8.
NO = int(os.environ.get("BITLIN_NO", "8"))
NM = N_CORES // NO
STRATEGY = os.environ.get("BITLIN_STRATEGY", "f32r")

MT = 512  # moving free dim per matmul (max for 4-byte dtypes, 1 psum bank)
P = 128


def _build(o_c: int, m_c: int, i_dim: int, strategy: str):
    """Build + compile the per-core Bass program.

    o_c: out-features per core, m_c: tokens per core, i_dim: contraction dim.
    DRAM io: w [o_c, i_dim] f32, xt [i_dim, m_c] f32, outt [o_c, m_c] f32.
    """
    dt = mybir.dt
    obs = o_c // P        # 128-row out-feature blocks
    kbs = i_dim // P      # contraction tiles
    mts = m_c // MT       # moving tiles
    mm_dt = {"f32r": dt.float32r, "bf16x2": dt.bfloat16, "bf16": dt.bfloat16}[strategy]

    nc = bacc.Bacc(
        "TRN2", target_bir_lowering=False, debug=False, num_devices=N_CORES
    )
    w_dram = nc.dram_tensor("w", [o_c, i_dim], dt.float32, kind="ExternalInput").ap()
    xt_dram = nc.dram_tensor("xt", [i_dim, m_c], dt.float32, kind="ExternalInput").ap()
    out_dram = nc.dram_tensor("outt", [o_c, m_c], dt.float32, kind="ExternalOutput").ap()

    with tile.TileContext(nc) as tc:
        with (
            tc.tile_pool(name="tt", bufs=1) as ttpool,      # resident lhsT tiles
            tc.tile_pool(name="const", bufs=1) as cpool,    # identity + scales
        ):
            ident = cpool.tile([P, P], dt.bfloat16, tag="ident")
            make_identity(nc, ident)

            scales = []
            tT = [[None] * kbs for _ in range(obs)]

            # ---- phase A: ternarize + transpose ----
            # absmean must effectively match the fp64 value: any ulp-level
            # deviation can flip a ternary decision for a weight sitting on
            # the 0.7*absmean boundary (~1e-2 output absmax error per flip).
            # Two-stage fp32 reduce for mean0, then one residual pass:
            #   absmean = mean0 + sum(|w| - mean0)/N
            # The residual sum has ~sqrt(N)*mean-magnitude cancellation, so
            # the corrected absmean lands within ~1e-9 relative of fp64.
            with (
                tc.tile_pool(name="wk", bufs=2) as wpool,
                tc.tile_pool(name="aw", bufs=1) as awpool,
                tc.tile_pool(name="stat", bufs=2) as spool,
                tc.tile_pool(name="tern", bufs=1) as tpool,
                tc.tile_pool(name="pst", bufs=4, space="PSUM") as pstpool,
            ):
                for ob in range(obs):
                    wsb = wpool.tile([P, i_dim], dt.float32, tag="wsb")
                    nc.sync.dma_start(out=wsb[:], in_=w_dram[ob * P:(ob + 1) * P, :])

                    aw = awpool.tile([P, i_dim], dt.float32, tag="aw")
                    nc.scalar.activation(
                        aw[:], wsb[:], mybir.ActivationFunctionType.Abs
                    )
                    aw3 = aw[:].rearrange("p (c k) -> p c k", k=P)
                    part = spool.tile([P, i_dim // P], dt.float32, tag="part")
                    nc.vector.tensor_reduce(
                        part[:], aw3, axis=mybir.AxisListType.X,
                        op=mybir.AluOpType.add,
                    )
                    s0 = spool.tile([P, 1], dt.float32, tag="s0")
                    nc.vector.tensor_reduce(
                        s0[:], part[:], axis=mybir.AxisListType.X,
                        op=mybir.AluOpType.add,
                    )
                    mean0 = spool.tile([P, 1], dt.float32, tag="mean0")
                    nc.vector.tensor_scalar_mul(mean0[:], s0[:], 1.0 / i_dim)
                    # residual pass (in-place into aw)
                    nc.vector.tensor_scalar(
                        aw[:], aw[:], mean0[:], None, mybir.AluOpType.subtract
                    )
                    rpart = spool.tile([P, i_dim // P], dt.float32, tag="rpart")
                    nc.vector.tensor_reduce(
                        rpart[:], aw3, axis=mybir.AxisListType.X,
                        op=mybir.AluOpType.add,
                    )
                    s1 = spool.tile([P, 1], dt.float32, tag="s1")
                    nc.vector.tensor_reduce(
                        s1[:], rpart[:], axis=mybir.AxisListType.X,
                        op=mybir.AluOpType.add,
                    )
                    # absmean = s1/N + mean0 ; also the output scale
                    scale = cpool.tile([P, 1], dt.float32, tag=f"scale{ob}")
                    nc.vector.tensor_scalar(
                        scale[:], s1[:], 1.0 / i_dim, mean0[:],
                        mybir.AluOpType.mult, mybir.AluOpType.add,
                    )
                    thr = spool.tile([P, 1], dt.float32, tag="thr")
                    nc.vector.tensor_scalar_mul(thr[:], scale[:], ALPHA)
                    nthr = spool.tile([P, 1], dt.float32, tag="nthr")
                    nc.vector.tensor_scalar_mul(nthr[:], scale[:], -ALPHA)

                    pos = tpool.tile([P, i_dim], dt.bfloat16, tag="pos")
                    nc.vector.tensor_scalar(
                        pos[:], wsb[:], thr[:], None, mybir.AluOpType.is_gt
                    )
                    neg = tpool.tile([P, i_dim], dt.bfloat16, tag="neg")
                    nc.vector.tensor_scalar(
                        neg[:], wsb[:], nthr[:], None, mybir.AluOpType.is_lt
                    )
                    tern = tpool.tile([P, i_dim], dt.bfloat16, tag="tern")
                    nc.vector.tensor_tensor(
                        tern[:], pos[:], neg[:], mybir.AluOpType.subtract
                    )
                    scales.append(scale)

                    for kb in range(kbs):
                        pst = pstpool.tile([P, P], dt.bfloat16, tag="pst")
                        nc.tensor.transpose(
                            pst[:], tern[:, kb * P:(kb + 1) * P], ident[:]
                        )
                        t = ttpool.tile([P, P], mm_dt, tag=f"t{ob}_{kb}")
                        nc.vector.tensor_copy(t[:], pst[:])
                        tT[ob][kb] = t

            # ---- phase B: stream x, matmul, scale, store ----
            with (
                tc.tile_pool(name="xf", bufs=6) as xfpool,
                tc.tile_pool(name="xr", bufs=10) as xrpool,
                tc.tile_pool(name="osb", bufs=6) as opool,
                tc.tile_pool(name="ps", bufs=(2 if obs <= 4 else 1), space="PSUM") as pspool,
            ):
                for mt in range(mts):
                    xr = []
                    xlo = []
                    for kb in range(kbs):
                        xf = xfpool.tile([P, MT], dt.float32, tag="xf")
                        nc.sync.dma_start(
                            out=xf[:],
                            in_=xt_dram[kb * P:(kb + 1) * P, mt * MT:(mt + 1) * MT],
                        )
                        xh = xrpool.tile([P, MT], mm_dt, tag="xr")
                        nc.vector.tensor_copy(xh[:], xf[:])
                        xr.append(xh)
                        if strategy == "bf16x2":
                            xl = xrpool.tile([P, MT], mm_dt, tag="xlo")
                            nc.vector.tensor_tensor(
                                xl[:], xf[:], xh[:], mybir.AluOpType.subtract
                            )
                            xlo.append(xl)

                    psum = []
                    for ob in range(obs):
                        pt = pspool.tile([P, MT], dt.float32, tag=f"ps{ob}")
                        psum.append(pt)
                    for kb in range(kbs):
                        last = kb == kbs - 1
                        for ob in range(obs):
                            if strategy == "bf16x2":
                                nc.tensor.matmul(
                                    psum[ob][:], tT[ob][kb][:], xr[kb][:],
                                    start=(kb == 0), stop=False,
                                )
                                nc.tensor.matmul(
                                    psum[ob][:], tT[ob][kb][:], xlo[kb][:],
                                    start=False, stop=last,
                                )
                            else:
                                nc.tensor.matmul(
                                    psum[ob][:], tT[ob][kb][:], xr[kb][:],
                                    start=(kb == 0), stop=last,
                                )

                    for ob in range(obs):
                        osb = opool.tile([P, MT], dt.float32, tag="osb")
                        nc.scalar.activation(
                            osb[:], psum[ob][:],
                            mybir.ActivationFunctionType.Copy,
                            scale=scales[ob][:],
                        )
                        nc.sync.dma_start(
                            out=out_dram[ob * P:(ob + 1) * P, mt * MT:(mt + 1) * MT],
                            in_=osb[:],
                        )

    nc.compile()
    return nc


_CACHE: dict = {}


def _get_nc(o_c, m_c, i_dim, strategy):
    key = (o_c, m_c, i_dim, strategy)
    if key not in _CACHE:
        _CACHE[key] = _build(o_c, m_c, i_dim, strategy)
    return _CACHE[key]


def _run(x2d: np.ndarray, weight: np.ndarray, no: int, nm: int, strategy: str,
         **run_kwargs):
    """x2d [M, I] f32, weight [O, I] f32 -> out [M, O] f32."""
    m, i_dim = x2d.shape
    o = weight.shape[0]
    o_c, m_c = o // no, m // nm
    nc = _get_nc(o_c, m_c, i_dim, strategy)

    xt = np.ascontiguousarray(x2d.T)  # [I, M]
    in_maps = []
    for c in range(no * nm):
        io, im = c // nm, c % nm
        in_maps.append({
            "w": np.ascontiguousarray(weight[io * o_c:(io + 1) * o_c]),
            "xt": xt if nm == 1 else np.ascontiguousarray(
                xt[:, im * m_c:(im + 1) * m_c]),
        })
    res = run_bass_kernel_spmd(nc, in_maps, core_ids=list(range(no * nm)),
                               **run_kwargs)
    outT = np.empty((o, m), dtype=np.float32)
    for c in range(no * nm):
        io, im = c // nm, c % nm
        outT[io * o_c:(io + 1) * o_c, im * m_c:(im + 1) * m_c] = \
            res.results[c]["outt"]
    out = np.ascontiguousarray(outT.T)  # [M, O]
    return out, res


def kernel(x: np.ndarray, weight: np.ndarray) -> np.ndarray:
    x = np.asarray(x, dtype=np.float32)
    weight = np.asarray(weight, dtype=np.float32)
    b, s, i_dim = x.shape
    out, _ = _run(x.reshape(b * s, i_dim), weight, NO, NM, STRATEGY)
    return out.reshape(b, s, weight.shape[0])
